# revision 11
# baseline (speedup 1.0000x reference)
"""Trainium2 Bass kernel for nn_EnsemblesWithMessagePassing.

Strategy: data-parallel over token positions (shard N=512 across the 8
NeuronCores, 64 positions each => 128 (b,n) tokens per core). The voting
attention is strictly per-position over the M=16 local messages, so this
sharding needs no collectives.

Key optimization vs the original: the member Linear is algebraically
fused into the kv projection of the output messages on the CPU:
    kv_{L+l} = (x_l @ wnet_l^T) @ wkv = x_l @ (wnet_l^T @ wkv)
so the on-chip member Linear matmuls, PE transposes of the Linear output
and their PSUM->SBUF copies all disappear; the out-message kv becomes a
plain matmul with the prefused weight (same DMA bytes, 40% fewer PE rows).

Engine budget (measured): DVE ~0.67ns/elem bf16 tensor_tensor,
~1.4ns/elem tensor_reduce, GpSimd ~3x slower than DVE, ACT ~0.85ns/elem
unary. The per-token attention element math is the bottleneck, so:
  - squares for rms stats go to ACT (Square activation)
  - k^2 (for k-rmsnorm) goes to GpSimd
  - sim/o products+trees stay on DVE, except 2 of 8 members' o-trees on
    GpSimd to balance
  - gates matmuls are emitted after the kv loop so they fill PE slack
    during the DVE-bound attention tail
  - reductions are tree-adds (TT rate) with a final 16->1 reduce
"""
import sys

for _p in ("/opt/trn_rl_repo", "/root/.axon_site/_ro/trn_rl_repo"):
    if _p not in sys.path:
        sys.path.insert(0, _p)

try:  # NTFF profile hook glue (only needed if tracing is requested)
    import antenv.axon_hooks  # noqa: F401
except Exception:
    pass

from contextlib import ExitStack

import numpy as np

import concourse.bass as bass  # noqa: F401
import concourse.tile as tile
from concourse import bacc, mybir
from concourse import bass_utils
from concourse.masks import make_identity

f32 = mybir.dt.float32
bf16 = mybir.dt.bfloat16
AF = mybir.ActivationFunctionType
AL = mybir.AluOpType
AX = mybir.AxisListType

# problem shape
L, B, N, D = 8, 2, 512, 1024
H, DH = 8, 64
INNER = H * DH          # 512
M = 2 * L               # 16 messages
SCALE = DH ** -0.5
EPS = float(np.finfo(np.float32).eps)

NCORES = 8
NSL = N // NCORES       # 64 positions per core per batch row
T = B * NSL             # 128 tokens per core
LT = L * T              # 1024
DT = D // 128           # 8 d-tiles
IT = INNER // 128       # 4 inner-tiles

GPS_O_MEMBERS = (6, 7)  # members whose o-tree runs on GpSimd

_NC_CACHE = {}


def _build():
    adt = bf16
    hdt = bf16

    nc = bacc.Bacc("TRN2", target_bir_lowering=False, debug=False,
                   enable_asserts=False, num_devices=NCORES)

    xTb_d = nc.dram_tensor("xTb", [128, DT, LT], hdt, kind="ExternalInput").ap()
    wkv_d = nc.dram_tensor("wkvT", [128, DT, 2 * INNER], hdt, kind="ExternalInput").ap()
    wf_d = nc.dram_tensor("wfT", [L, 128, DT, 2 * INNER], hdt, kind="ExternalInput").ap()
    wq_d = nc.dram_tensor("wqT", [128, DT, INNER], hdt, kind="ExternalInput").ap()
    wg_d = nc.dram_tensor("wgT", [128, DT, H], hdt, kind="ExternalInput").ap()
    wout_d = nc.dram_tensor("woutT", [128, IT, D], adt, kind="ExternalInput").ap()
    onesc_d = nc.dram_tensor("onesc", [128, 2], hdt, kind="ExternalInput").ap()
    out_d = nc.dram_tensor("out", [L, T, D], adt, kind="ExternalOutput").ap()

    with tile.TileContext(nc) as tc, ExitStack() as ctx, \
            nc.allow_low_precision(
                reason="attention intermediates are <=64-term reductions in "
                       "bf16; verified rel err ~5e-3 vs fp32 reference"):
        pc = ctx.enter_context(tc.tile_pool(name="const", bufs=1))
        pb = ctx.enter_context(tc.tile_pool(name="big", bufs=1))
        pa = ctx.enter_context(tc.tile_pool(name="attp", bufs=1))
        pwf = ctx.enter_context(tc.tile_pool(name="wfp", bufs=2))
        psk = ctx.enter_context(tc.tile_pool(name="kp", bufs=2))
        pscr = ctx.enter_context(tc.tile_pool(name="scrp", bufs=2))
        pes = ctx.enter_context(tc.tile_pool(name="tailp", bufs=2))
        pgps = ctx.enter_context(tc.tile_pool(name="gpsp", bufs=1))
        pmm = ctx.enter_context(tc.tile_pool(name="psmm", bufs=4, space="PSUM"))
        ptp = ctx.enter_context(tc.tile_pool(name="pstp", bufs=2, space="PSUM"))
        psm = ctx.enter_context(tc.tile_pool(name="pssm", bufs=2, space="PSUM"))

        # ---- constants ----
        ident_a = pc.tile([128, 128], adt, tag="ident_a")
        make_identity(nc, ident_a)
        onesc = pc.tile([128, 2], hdt, tag="onesc")
        nc.sync.dma_start(onesc[:], onesc_d[:])
        ones_f = pc.tile([1, 2], f32, tag="ones_f")
        nc.vector.memset(ones_f[:], 1.0)
        eps_c = pc.tile([128, 1], f32, tag="eps")
        nc.vector.memset(eps_c[:], EPS)

        # ---- input / weight loads (SP queue order == emission order) ----
        xTb = pb.tile([128, DT, LT], hdt, tag="xTb")
        nc.sync.dma_start(xTb[:], xTb_d[:])
        wq = pb.tile([128, DT, INNER], hdt, tag="wq")
        nc.sync.dma_start(wq[:], wq_d[:])
        wkv = pb.tile([128, DT, 2 * INNER], hdt, tag="wkv")
        nc.sync.dma_start(wkv[:], wkv_d[:])
        wg = pb.tile([128, DT, H], hdt, tag="wg")
        nc.sync.dma_start(wg[:], wg_d[:])

        # whole-kernel attention state
        q_all = pa.tile([128, L, INNER], adt, tag="q_all")
        v_allT = pa.tile([128, H, DH, M], adt, tag="v_allT")
        g_all = pa.tile([128, L, H], f32, tag="g_all")
        kss_all = pa.tile([128, M, H], f32, tag="kss_all")
        sim_all = pa.tile([128, L, H, M], adt, tag="sim_all")

        # ---- phase A: token rms stats (squares on ACT) ----
        with nc.named_scope("stats"):
            rms_row = pb.tile([1, LT], f32, tag="rms_row")
            for c in range(2):
                ssps = psm.tile([2, 512], f32, tag="sm")
                for d in range(DT):
                    sq = psk.tile([128, 512], hdt, tag="sq")
                    nc.scalar.square(sq[:], xTb[:, d, c * 512:(c + 1) * 512])
                    nc.tensor.matmul(ssps[:], onesc[:], sq[:],
                                     start=(d == 0), stop=(d == DT - 1))
                nc.scalar.activation(rms_row[0:1, c * 512:(c + 1) * 512],
                                     ssps[0:1, :], AF.Sqrt,
                                     scale=1.0 / D, bias=eps_c[0:1, 0:1])
            # move to token-major [t, l] first, then one wide reciprocal
            rsps = psm.tile([128, L], f32, tag="sm")
            for l in range(L):
                nc.tensor.matmul(rsps[:, l:l + 1],
                                 rms_row[0:1, l * T:(l + 1) * T],
                                 ones_f[0:1, 0:1], start=True, stop=True)
            rs_tok = pb.tile([128, L], f32, tag="rs_tok")
            nc.vector.reciprocal(rs_tok[:], rsps[:])

        # ---- phase B: queries (gates deferred to fill PE slack later) ----
        with nc.named_scope("qg"):
            for l in range(L):
                qps = pmm.tile([128, INNER], f32, tag="mm")
                for d in range(DT):
                    nc.tensor.matmul(qps[:], xTb[:, d, l * T:(l + 1) * T],
                                     wq[:, d], start=(d == 0),
                                     stop=(d == DT - 1))
                nc.scalar.activation(q_all[:, l], qps[:], AF.Copy,
                                     scale=rs_tok[:, l:l + 1])

        # ---- phase C: kv + sim products per member ----
        with nc.named_scope("linkv"):
            for l in range(L):
                wf = pwf.tile([128, DT, 2 * INNER], hdt, tag="wf")
                nc.sync.dma_start(wf[:], wf_d[l])
                for src, m in ((wkv, l), (wf, L + l)):
                    for half in range(2):  # 0 = k, 1 = v
                        ps = pmm.tile([128, INNER], f32, tag="mm")
                        for d in range(DT):
                            nc.tensor.matmul(
                                ps[:], xTb[:, d, l * T:(l + 1) * T],
                                src[:, d, half * INNER:(half + 1) * INNER],
                                start=(d == 0), stop=(d == DT - 1))
                        if half == 0:
                            k_m = psk.tile([128, INNER], adt, tag="k_m")
                            nc.scalar.copy(k_m[:], ps[:])
                            ksq = psk.tile([128, INNER], f32, tag="ksq")
                            nc.scalar.square(ksq[:], ps[:])
                            nc.vector.tensor_reduce(
                                kss_all[:, m],
                                ksq.rearrange("p (h d) -> p h d", d=DH),
                                axis=AX.X, op=AL.add)
                            # sim products: all 8 queries vs this message
                            scr = pscr.tile([128, L, INNER], adt, tag="scr")
                            nc.vector.tensor_tensor(
                                scr[:], q_all[:],
                                k_m[:, None].to_broadcast([128, L, INNER]),
                                AL.mult)
                            sv = scr.rearrange("p l (h d) -> p l h d", d=DH)
                            nc.vector.tensor_tensor(
                                sv[:, :, :, 0:DH // 2], sv[:, :, :, 0:DH // 2],
                                sv[:, :, :, DH // 2:DH], AL.add)
                            nc.vector.tensor_tensor(
                                sv[:, :, :, 0:DH // 4], sv[:, :, :, 0:DH // 4],
                                sv[:, :, :, DH // 4:DH // 2], AL.add)
                            nc.vector.tensor_reduce(
                                sim_all[:, :, :, m], sv[:, :, :, 0:DH // 4],
                                axis=AX.X, op=AL.add)
                        else:
                            nc.scalar.copy(
                                v_allT[:, :, :, m],
                                ps.rearrange("p (h d) -> p h d", d=DH))

        # ---- gates: emitted late so the matmuls run during the DVE tail ----
        with nc.named_scope("gates"):
            for l in range(L):
                gps = psm.tile([128, H], f32, tag="sm")
                for d in range(DT):
                    nc.tensor.matmul(gps[:], xTb[:, d, l * T:(l + 1) * T],
                                     wg[:, d], start=(d == 0),
                                     stop=(d == DT - 1))
                nc.scalar.activation(g_all[:, l], gps[:], AF.Sigmoid,
                                     scale=rs_tok[:, l:l + 1])

        # ---- tail weights ----
        wout = pb.tile([128, IT, D], adt, tag="wout")
        nc.sync.dma_start(wout[:], wout_d[:])

        # ---- phase E: batched softmax + per-member o / pooled ----
        with nc.named_scope("attn"):
            krms = pes.tile([128, M, H], f32, tag="krms")
            nc.scalar.activation(krms.rearrange("p m h -> p (m h)"),
                                 kss_all.rearrange("p m h -> p (m h)"),
                                 AF.Sqrt, scale=1.0 / DH, bias=eps_c[:, 0:1])
            krinv = pes.tile([128, M, H], f32, tag="krinv")
            nc.vector.reciprocal(krinv.rearrange("p m h -> p (m h)"),
                                 krms.rearrange("p m h -> p (m h)"))
            nc.vector.tensor_tensor(
                sim_all[:], sim_all[:],
                krinv.rearrange("p m h -> p h m")[:, None]
                .to_broadcast([128, L, H, M]), AL.mult)
            mx_all = pes.tile([128, L, H], adt, tag="mx_all")
            nc.vector.tensor_reduce(mx_all[:], sim_all[:], axis=AX.X,
                                    op=AL.max)
            nc.vector.tensor_tensor(
                sim_all[:], sim_all[:],
                mx_all[:, :, :, None].to_broadcast([128, L, H, M]),
                AL.subtract)
            pl_all = pes.tile([128, L, H, M], adt, tag="pl_all")
            nc.scalar.activation(pl_all.rearrange("p l h m -> p (l h m)"),
                                 sim_all.rearrange("p l h m -> p (l h m)"),
                                 AF.Exp)
            sm_all = pes.tile([128, L, H], f32, tag="sm_all")
            nc.vector.tensor_reduce(sm_all[:], pl_all[:], axis=AX.X,
                                    op=AL.add)
            rgf = pes.tile([128, L, H], f32, tag="rgf")
            nc.vector.reciprocal(rgf.rearrange("p l h -> p (l h)"),
                                 sm_all.rearrange("p l h -> p (l h)"))
            rg = pes.tile([128, L, H], adt, tag="rg")
            nc.vector.tensor_tensor(rg[:], rgf[:], g_all[:], AL.mult)
            nc.vector.tensor_tensor(
                pl_all[:], pl_all[:],
                rg[:, :, :, None].to_broadcast([128, L, H, M]), AL.mult)
            half, quart = M // 2, M // 4
            # GpSimd members first so their slow chains overlap the DVE ones
            order = list(GPS_O_MEMBERS) + [l for l in range(L)
                                           if l not in GPS_O_MEMBERS]
            for l in order:
                gps = l in GPS_O_MEMBERS
                eng = nc.gpsimd if gps else nc.vector
                prod = (pgps if gps else pscr).tile([128, H, DH, M], adt,
                                                    tag="prod")
                eng.tensor_tensor(
                    prod[:],
                    pl_all[:, l, :, None, :].to_broadcast([128, H, DH, M]),
                    v_allT[:], AL.mult)
                eng.tensor_tensor(
                    prod[:, :, :, 0:half], prod[:, :, :, 0:half],
                    prod[:, :, :, half:M], AL.add)
                eng.tensor_tensor(
                    prod[:, :, :, 0:quart], prod[:, :, :, 0:quart],
                    prod[:, :, :, quart:half], AL.add)
                o_l = pes.tile([128, INNER], adt, tag="o_l")
                nc.vector.tensor_reduce(
                    o_l.rearrange("p (h d) -> p h d", d=DH),
                    prod[:, :, :, 0:quart], axis=AX.X, op=AL.add)
                # pooled = o @ wout
                oTt = pes.tile([128, IT, 128], adt, tag="oTt")
                for it in range(IT):
                    tps = ptp.tile([128, 128], adt, tag="tp")
                    nc.tensor.transpose(tps[:], o_l[:, it * 128:(it + 1) * 128],
                                        ident_a[:])
                    nc.scalar.copy(oTt[:, it], tps[:])
                pout = pes.tile([128, D], adt, tag="pout")
                for oc in range(2):
                    ps = pmm.tile([128, 512], f32, tag="mm")
                    for it in range(IT):
                        nc.tensor.matmul(ps[:], oTt[:, it],
                                         wout[:, it, oc * 512:(oc + 1) * 512],
                                         start=(it == 0), stop=(it == IT - 1))
                    nc.scalar.copy(pout[:, oc * 512:(oc + 1) * 512], ps[:])
                nc.sync.dma_start(out_d[l][:], pout[:])

    nc.compile()
    return nc


def get_nc():
    if "nc" not in _NC_CACHE:
        _NC_CACHE["nc"] = _build()
    return _NC_CACHE["nc"]


def prep_weights(w_net, b_net, norm_w, wq, wkv, knorm_w, wg, wout):
    """CPU-side layout prep shared by all cores."""
    import ml_dtypes
    bf = ml_dtypes.bfloat16
    # fused out-message kv weight: wf_l = wnet_l^T @ wkv  [D, 2*INNER]
    # (torch Linear weight is [out, in]; out = x @ wnet^T, kv_out = out @ wkv)
    wf = np.einsum('lod,ok->ldk', w_net, wkv, optimize=True)  # [L, D, 2I]
    if np.any(b_net):
        raise NotImplementedError("nonzero b_net not supported by this kernel")
    wfT = np.ascontiguousarray(
        wf.reshape(L, DT, 128, 2 * INNER).transpose(0, 2, 1, 3))
    colscale = (np.tile(knorm_w, H) * SCALE).astype(np.float32)
    wq2 = norm_w[:, None] * wq * colscale[None, :]
    wqT = np.ascontiguousarray(wq2.reshape(DT, 128, INNER).transpose(1, 0, 2))
    wkvT = np.ascontiguousarray(wkv.reshape(DT, 128, 2 * INNER).transpose(1, 0, 2))
    wgT = np.ascontiguousarray((norm_w[:, None] * wg).reshape(DT, 128, H)
                               .transpose(1, 0, 2))
    woutT = np.ascontiguousarray(wout.reshape(IT, 128, D).transpose(1, 0, 2))
    return dict(
        wfT=wfT.astype(bf),
        wqT=wqT.astype(bf),
        wkvT=wkvT.astype(bf),
        wgT=wgT.astype(bf),
        woutT=woutT.astype(bf),
        onesc=np.ones((128, 2), dtype=bf),
    )


def prep_core_x(tokens, c):
    """Per-core feature-major token slice: [128, DT, LT]."""
    xs = tokens[:, :, c * NSL:(c + 1) * NSL, :].reshape(L, T, D)
    xT = xs.reshape(L, T, DT, 128).transpose(3, 2, 0, 1).reshape(128, DT, LT)
    return np.ascontiguousarray(xT)


def make_in_maps(tokens, w_net, b_net, norm_w, wq, wkv, knorm_w, wg, wout):
    shared = prep_weights(np.asarray(w_net, np.float32), np.asarray(b_net, np.float32),
                          np.asarray(norm_w, np.float32), np.asarray(wq, np.float32),
                          np.asarray(wkv, np.float32), np.asarray(knorm_w, np.float32),
                          np.asarray(wg, np.float32), np.asarray(wout, np.float32))
    import ml_dtypes
    tokens = np.asarray(tokens, np.float32)
    maps = []
    for c in range(NCORES):
        xT = prep_core_x(tokens, c)
        maps.append(dict(shared, xTb=xT.astype(ml_dtypes.bfloat16)))
    return maps


def stitch(results):
    full = np.empty((L, B, N, D), dtype=np.float32)
    for c in range(NCORES):
        full[:, :, c * NSL:(c + 1) * NSL, :] = \
            results[c]["out"].astype(np.float32).reshape(L, B, NSL, D)
    return full


def kernel(tokens, w_net, b_net, norm_w, wq, wkv, knorm_w, wg, wout):
    nc = get_nc()
    in_maps = make_in_maps(tokens, w_net, b_net, norm_w, wq, wkv, knorm_w, wg, wout)
    res = bass_utils.run_bass_kernel_spmd(nc, in_maps, core_ids=list(range(NCORES)))
    return stitch(res.results)
